# revision 73
# baseline (speedup 1.0000x reference)
"""BiSTSSM (bidirectional Mamba-style selective scan) on 8 Trainium2 cores.

Sharding: core c = (b, k) for the scan launch (B=4 batches x K=2 directions);
core c = (b, t-half) for the merge+LayerNorm launch.

Scan layout: partitions hold (64 d-channels x 2 states), so each 128-channel
d-tile is two 64-channel blocks (hb) of 8 state-groups (g) each:
    p = 2*d_loc + n_loc,  n = 2*g + n_loc.
Engine split per (i, hb, g):
    Act : da = exp(A * delta)         (table op, only engine with exp)
    DVE : selective-scan recurrence   (tensor_tensor_scan, DVE-only op)
    DVE/Pool : dBu = du*B and h*C muls, split to balance the two engines
    PE  : block-diagonal n-reduction y += s2.T @ (h*C)
"""

import numpy as np
from contextlib import ExitStack

import concourse.bass as bass
import concourse.mybir as mybir
import concourse.tile as tile
from concourse import bacc
from concourse import hw_specs as _hw_specs
from concourse.masks import make_identity
from concourse.bass_utils import run_bass_kernel_spmd

# Steer the ACT table-set chooser away from exp-only sets so Exp and Ln both
# resolve to natural_log_exp_and_others (one table load for the whole kernel
# instead of a load per Exp<->Ln transition). Ids are positional, so the
# competing sets are emptied rather than removed.
_orig_get_tables = _hw_specs.get_activation_tables
_ACT_KEEP = {"natural_log_exp_and_others"}


def _patched_get_tables(module_arch):
    tables = dict(_orig_get_tables(module_arch))
    for name in tables:
        if name not in _ACT_KEEP:
            tables[name] = set()
    return tables


_hw_specs.get_activation_tables = _patched_get_tables
import concourse.bacc as _bacc_mod
if hasattr(_bacc_mod, "get_activation_tables"):
    _bacc_mod.get_activation_tables = _patched_get_tables

F = mybir.dt.float32
H16 = mybir.dt.float16
OP = mybir.AluOpType
AF = mybir.ActivationFunctionType

D_INNER, J, N, R, K = 384, 5, 16, 12, 2
B, T = 4, 2048
D = D_INNER * J          # 1920 scan channels
NT = D // 128            # 15 d-tiles of 128
TH = T // 2              # half-T (phase A chunking; merge-launch T half)
NG = 8                   # state-groups of 2 states each (8 x 2 = 16)


def _build_scan_nc():
    global _ACT_KEEP
    _ACT_KEEP = {"natural_log_exp_and_others"}
    nc = bacc.Bacc()
    x_d = nc.dram_tensor("x", [D, T], H16, kind="ExternalInput")
    wxT_d = nc.dram_tensor("wxT", [128, NT * 80], H16, kind="ExternalInput")
    wdtT_d = nc.dram_tensor("wdtT", [R, D], H16, kind="ExternalInput")
    bias_d = nc.dram_tensor("bias_r", [128, NT], F, kind="ExternalInput")
    alog_d = nc.dram_tensor("alog_r", [128, NT * 2 * NG], F, kind="ExternalInput")
    s2_d = nc.dram_tensor("s2", [128, 256], H16, kind="ExternalInput")
    dsc_d = nc.dram_tensor("dsc", [128, NT], F, kind="ExternalInput")
    ys_d = nc.dram_tensor("ys", [D, T], H16, kind="ExternalOutput")

    dl_drams = [nc.dram_tensor(f"dl_scr{i}", [128, T], H16) for i in range(NT)]
    du_drams = [nc.dram_tensor(f"du_scr{i}", [128, T], H16) for i in range(NT)]
    bc_dram = nc.dram_tensor("bc_scr", [N, T], H16)
    cc_dram = nc.dram_tensor("cc_scr", [N, T], H16)

    C44 = 80  # padded proj rows: dtr 0:12, B 32:48, C 64:80

    with tile.TileContext(nc) as tc, ExitStack() as ctx:
        singles = ctx.enter_context(tc.tile_pool(name="singles", bufs=1))
        xpool = ctx.enter_context(tc.tile_pool(name="xpool", bufs=1))
        wpool = ctx.enter_context(tc.tile_pool(name="wpool", bufs=1))
        dpool = ctx.enter_context(tc.tile_pool(name="dpool", bufs=2))
        spool = ctx.enter_context(tc.tile_pool(name="spool", bufs=2))
        ypool = ctx.enter_context(tc.tile_pool(name="ypool", bufs=2))
        pwork = ctx.enter_context(tc.tile_pool(name="pwork", bufs=1, space="PSUM"))
        pyq = ctx.enter_context(tc.tile_pool(name="pyq", bufs=3, space="PSUM"))

        # --- constants (loads emitted after the first x-stream below) ---
        s2_t = singles.tile([128, 256], H16)
        bias_t = singles.tile([128, NT], F)
        alog_t = singles.tile([128, NT * 2 * NG], F)
        a_t = singles.tile([128, NT * 2 * NG], F)   # A = -exp(A_logs)
        wdtT_t = singles.tile([R, D], H16)
        wt_t = singles.tile([128, NT * 80], H16)
        dsc_t = singles.tile([128, NT], F)
        idn_t = singles.tile([128, 128], H16)

        def load_consts():
            nc.sync.dma_start(bias_t[:], bias_d[:, :])
            nc.sync.dma_start(alog_t[:], alog_d[:, :])
            nc.sync.dma_start(s2_t[:], s2_d[:, :])
            nc.scalar.activation(a_t[:], alog_t[:], AF.Exp)
            nc.vector.tensor_scalar_mul(a_t[:], a_t[:], -1.0)
            nc.sync.dma_start(wdtT_t[:], wdtT_d[:, :])
            nc.sync.dma_start(dsc_t[:], dsc_d[:, :])
            make_identity(nc, idn_t[:])
        dtr_ts = [singles.tile([R, TH], H16, name=f"dtr{h}") for h in range(2)]
        bc_ts = [singles.tile([N, TH], H16, name=f"bch{h}") for h in range(2)]
        cc_ts = [singles.tile([N, TH], H16, name=f"cch{h}") for h in range(2)]
        bb_ts = [singles.tile([128, 2 * T], H16, name=f"bcast_b{gq}")
                 for gq in range(NG // 2)]
        cb_ts = [singles.tile([128, 2 * T], H16, name=f"bcast_c{gq}")
                 for gq in range(NG // 2)]
        wts = []

        # ---- phase A emitters ----
        def a_prologue(h):
            hsl = slice(h * TH, (h + 1) * TH)
            xdbl_p = pwork.tile([C44, TH], F, tag="work", name=f"xdbl_{h}")
            if h == 0:
                nc.sync.dma_start(wt_t[:], wxT_d[:, :])
            for i in range(NT):
                xt = xpool.tile([128, TH], H16, tag="xs", bufs=4)
                nc.sync.dma_start(xt[:], x_d[i * 128:(i + 1) * 128, hsl])
                for jj in range(TH // 512):
                    nc.tensor.matmul(
                        xdbl_p[:, jj * 512:(jj + 1) * 512],
                        wt_t[:, i * 80:(i + 1) * 80],
                        xt[:, jj * 512:(jj + 1) * 512],
                        start=(i == 0), stop=(i == NT - 1))
            if h == 0:
                load_consts()
            nc.scalar.copy(dtr_ts[h][:], xdbl_p[0:R, :])
            nc.scalar.copy(bc_ts[h][:], xdbl_p[32:32 + N, :])
            nc.scalar.copy(cc_ts[h][:], xdbl_p[64:64 + N, :])
            nc.sync.dma_start(bc_dram[:, hsl], bc_ts[h][:])
            nc.sync.dma_start(cc_dram[:, hsl], cc_ts[h][:])

        def a_tile(i, h):
            hsl = slice(h * TH, (h + 1) * TH)
            dts_p = pwork.tile([128, TH], F, tag="work")
            for jj in range(TH // 512):
                nc.tensor.matmul(
                    dts_p[:, jj * 512:(jj + 1) * 512],
                    wdtT_t[:, i * 128:(i + 1) * 128],
                    dtr_ts[h][:, jj * 512:(jj + 1) * 512],
                    start=True, stop=True)
            # delta = softplus(dts + bias) = ln(exp(dts + bias) + 1)
            de_t = dpool.tile([128, TH], H16, tag="de")
            nc.scalar.activation(de_t[:], dts_p[:], AF.Exp,
                                 bias=bias_t[:, i:i + 1])
            dl_t = dpool.tile([128, TH], H16, tag="dl")
            nc.scalar.activation(dl_t[:], de_t[:], AF.Ln, bias=1.0)
            nc.sync.dma_start(dl_drams[i][:, hsl], dl_t[:])
            # du = delta * u  (DVE: idle during phase A; x re-streamed)
            xt2 = xpool.tile([128, TH], H16, tag="xs2", bufs=2)
            nc.sync.dma_start(xt2[:], x_d[i * 128:(i + 1) * 128, hsl])
            du_t = dpool.tile([128, TH], H16, tag="du")
            nc.vector.tensor_mul(du_t[:], dl_t[:], xt2[:])
            nc.sync.dma_start(du_drams[i][:, hsl], du_t[:])

        def a_bcast_gq(hh, gq):
            # B/C broadcast into the (64d x 2n) layout; tile gq holds the
            # g-pair (2gq, 2gq+1): bb[p, gg*T + t] = B[2*(2gq+gg) + p%2, t]
            for src_dram, bt in ((bc_dram, bb_ts[gq]), (cc_dram, cb_ts[gq])):
                for gg in range(2):
                    g = 2 * gq + gg
                    src = bass.AP(tensor=src_dram[:, :].tensor,
                                  offset=2 * g * T + hh * TH,
                                  ap=[[0, 64], [T, 2], [1, TH]])
                    nc.sync.dma_start(
                        bt[:, gg * T + hh * TH:gg * T + (hh + 1) * TH],
                        src)

        # ---- phase C emitters (software-pipelined units) ----
        # unit u = (i, hb); stage1 = reload + dBu muls, stage2 = scans etc.
        unit_state = {}

        def c_stage1(u):
            i, hb = u
            dlb_t = spool.tile([128, T], H16, tag="dlb", bufs=2)
            dub_t = spool.tile([128, T], H16, tag="dub", bufs=2)
            for dr, dst in ((dl_drams[i], dlb_t), (du_drams[i], dub_t)):
                src = bass.AP(tensor=dr[:, :].tensor, offset=hb * 64 * T,
                              ap=[[T, 64], [0, 2], [1, T]])
                nc.sync.dma_start(dst[:], src)
            dub3 = bass.AP(tensor=dub_t[:].tensor, offset=dub_t[:].offset,
                           ap=[dub_t[:].ap[0], [0, 2], [1, T]])
            b16s = []
            for gq in range(NG // 2):
                # b16 = du * B for the g-pair (one 4096-elem mul, Pool)
                b16_t = spool.tile([128, 2 * T], H16, tag="b16", bufs=3)
                nc.gpsimd.tensor_mul(b16_t[:], dub3, bb_ts[gq][:])
                b16s.append(b16_t)
            unit_state[u] = (dlb_t, b16s)

        def skip_add_full(i, y_halves):
            # y += Dsum*u via PE identity matmul (zero column on k=1 cores)
            xu = xpool.tile([128, T], H16, tag="xu", bufs=2)
            nc.sync.dma_start(xu[:], x_d[i * 128:(i + 1) * 128, :])
            nc.scalar.mul(xu[:], xu[:], dsc_t[:, i:i + 1])
            for q in range(2):
                for c in range(2):
                    nc.tensor.matmul(
                        y_halves[q][:, c * 512:(c + 1) * 512], idn_t[:],
                        xu[:, q * TH + c * 512:q * TH + (c + 1) * 512],
                        start=True, stop=False)

        def skip_add(i, q, y_half, hslice):
            xu = xpool.tile([128, TH], H16, tag="xu", bufs=2)
            nc.sync.dma_start(xu[:], x_d[i * 128:(i + 1) * 128, hslice])
            nc.scalar.mul(xu[:], xu[:], dsc_t[:, i:i + 1])
            for c in range(2):
                nc.tensor.matmul(
                    y_half[:, c * 512:(c + 1) * 512], idn_t[:],
                    xu[:, c * 512:(c + 1) * 512], start=True, stop=False)

        def c_stage2(u, y_halves):
            i, hb = u
            dlb_t, b16s = unit_state.pop(u)
            if hb == 0:
                for q in range(2):
                    skip_add(i, q, y_halves[q], slice(q * TH, (q + 1) * TH))
            for gq in range(NG // 2):
                b16_t = b16s[gq]
                h16_t = spool.tile([128, 2 * T], H16, tag="h16", bufs=2)
                for gg in range(2):
                    g = 2 * gq + gg
                    da_t = spool.tile([128, T], H16, tag="da", bufs=3)
                    col = i * 16 + hb * 8 + g
                    nc.scalar.activation(da_t[:], dlb_t[:], AF.Exp,
                                         scale=a_t[:, col:col + 1])
                    nc.vector.tensor_tensor_scan(
                        h16_t[:, gg * T:(gg + 1) * T], da_t[:],
                        b16_t[:, gg * T:(gg + 1) * T],
                        0.0, op0=OP.mult, op1=OP.add)
                hc_t = spool.tile([128, 2 * T], H16, tag="hc", bufs=2)
                eng = nc.vector if gq in (1, 3) else nc.gpsimd
                eng.tensor_mul(hc_t[:], h16_t[:], cb_ts[gq][:])
                for jj in (0, 1, 4, 5, 2, 3, 6, 7):
                    tc512 = jj % 4
                    nc.tensor.matmul(
                        y_halves[tc512 // 2][:, (tc512 % 2) * 512:
                                             (tc512 % 2 + 1) * 512],
                        s2_t[:, hb * 128:(hb + 1) * 128],
                        hc_t[:, jj * 512:(jj + 1) * 512],
                        start=False,
                        stop=(hb == 1 and gq == 3 and jj >= 4))
            if hb == 1:
                for q in range(2):
                    ys_t = ypool.tile([128, TH], H16, tag="ys")
                    nc.scalar.copy(ys_t[:], y_halves[q][:])
                    nc.sync.dma_start(
                        ys_d[i * 128:(i + 1) * 128, q * TH:(q + 1) * TH],
                        ys_t[:])

        # ---- carry-split phase C for the first CARRY tiles: process each
        # (i, hb) unit in two half-T parts, so scans start while phase A is
        # still staging the other half. Only the [128, 1] scan state column
        # is carried between the parts.
        CARRY = 3
        carry_cols = {(i, hb): singles.tile([128, NG], H16, name=f"cc_{i}_{hb}")
                      for i in range(CARRY) for hb in range(2)}
        carry_y = {}

        def c_half(u, h, bcast_h=None):
            i, hb = u
            if hb == 0:
                carry_y[(i, h)] = pyq.tile([128, TH], F, tag="yq",
                                           name=f"yc_{i}_{h}")
                skip_add(i, h, carry_y[(i, h)],
                         slice(h * TH, (h + 1) * TH))
            y_ps = carry_y[(i, h)]
            dlbh = spool.tile([128, TH], H16, tag="dlb", bufs=2)
            dubh = spool.tile([128, TH], H16, tag="dub", bufs=2)
            for dr, dst in ((dl_drams[i], dlbh), (du_drams[i], dubh)):
                src = bass.AP(tensor=dr[:, :].tensor,
                              offset=hb * 64 * T + h * TH,
                              ap=[[T, 64], [0, 2], [1, TH]])
                nc.sync.dma_start(dst[:], src)
            dub3h = bass.AP(tensor=dubh[:].tensor, offset=dubh[:].offset,
                            ap=[dubh[:].ap[0], [0, 2], [1, TH]])
            for gq in range(NG // 2):
                if bcast_h is not None:
                    a_bcast_gq(bcast_h, gq)
                b16h = spool.tile([128, 2 * TH], H16, tag="b16", bufs=3)
                b16o = bass.AP(tensor=b16h[:].tensor, offset=b16h[:].offset,
                               ap=[b16h[:].ap[0], [TH, 2], [1, TH]])
                bbh = bass.AP(tensor=bb_ts[gq][:].tensor,
                              offset=bb_ts[gq][:].offset + h * TH,
                              ap=[bb_ts[gq][:].ap[0], [T, 2], [1, TH]])
                nc.gpsimd.tensor_mul(b16o, dub3h, bbh)
                h16h = spool.tile([128, 2 * TH], H16, tag="h16", bufs=2)
                for gg in range(2):
                    g = 2 * gq + gg
                    dah = spool.tile([128, TH], H16, tag="da", bufs=3)
                    col = i * 16 + hb * 8 + g
                    nc.scalar.activation(dah[:], dlbh[:], AF.Exp,
                                         scale=a_t[:, col:col + 1])
                    init = (0.0 if h == 0
                            else carry_cols[u][:, g:g + 1])
                    nc.vector.tensor_tensor_scan(
                        h16h[:, gg * TH:(gg + 1) * TH], dah[:],
                        b16h[:, gg * TH:(gg + 1) * TH],
                        init, op0=OP.mult, op1=OP.add)
                    if h == 0:
                        nc.gpsimd.tensor_copy(
                            carry_cols[u][:, g:g + 1],
                            h16h[:, gg * TH + TH - 1:gg * TH + TH])
                hch = spool.tile([128, 2 * TH], H16, tag="hc", bufs=2)
                hco = bass.AP(tensor=hch[:].tensor, offset=hch[:].offset,
                              ap=[hch[:].ap[0], [TH, 2], [1, TH]])
                h16o = bass.AP(tensor=h16h[:].tensor, offset=h16h[:].offset,
                               ap=[h16h[:].ap[0], [TH, 2], [1, TH]])
                cbh = bass.AP(tensor=cb_ts[gq][:].tensor,
                              offset=cb_ts[gq][:].offset + h * TH,
                              ap=[cb_ts[gq][:].ap[0], [T, 2], [1, TH]])
                heng = nc.vector if (h == 1 and gq in (1, 3)) else nc.gpsimd
                heng.tensor_mul(hco, h16o, cbh)
                for gg in range(2):
                    for tc in range(2):
                        nc.tensor.matmul(
                            y_ps[:, tc * 512:(tc + 1) * 512],
                            s2_t[:, hb * 128:(hb + 1) * 128],
                            hch[:, gg * TH + tc * 512:gg * TH + (tc + 1) * 512],
                            start=False,
                            stop=(hb == 1 and gq == 3 and gg == 1))
            if hb == 1:
                ys_t = ypool.tile([128, TH], H16, tag="ys")
                nc.scalar.copy(ys_t[:], y_ps[:])
                nc.sync.dma_start(
                    ys_d[i * 128:(i + 1) * 128, h * TH:(h + 1) * TH], ys_t[:])
                del carry_y[(i, h)]

        # ---- main schedule: interleave phase A with phase C ----
        units = [(i, hb) for i in range(CARRY, NT) for hb in range(2)]
        y_cur = {}
        s1ptr, s2ptr = [0], [0]

        def do_stage2(v):
            if v[1] == 0:
                y_cur[v[0]] = [
                    pyq.tile([128, TH], F, tag="yq", name=f"y_{v[0]}_{q}")
                    for q in range(2)]
            c_stage2(v, y_cur[v[0]])
            if v[1] == 1:
                del y_cur[v[0]]

        def advance(max_units):
            max_units = min(max_units, len(units))
            while s1ptr[0] < max_units:
                c_stage1(units[s1ptr[0]])
                s1ptr[0] += 1
                while s1ptr[0] - s2ptr[0] > 1:
                    do_stage2(units[s2ptr[0]])
                    s2ptr[0] += 1

        # h0 staging for tiles 0..7 only; tiles 8..14 are deferred into the
        # h1 loop so their DMAs don't clog the queue ahead of the phase-C
        # reloads at the h0->h1 transition.
        a_prologue(0)
        for i in range(8):
            a_tile(i, 0)
            if i < CARRY:
                c_half((i, 0), 0, bcast_h=0 if i == 0 else None)
                c_half((i, 1), 0)
        a_prologue(1)
        for i in range(NT):
            a_tile(i, 1)
            if i < CARRY:
                c_half((i, 0), 1, bcast_h=1 if i == 0 else None)
                c_half((i, 1), 1)
            else:
                advance(2 * (i - CARRY + 1))
            if i + 8 < NT:
                a_tile(i + 8, 0)
        advance(len(units))
        while s2ptr[0] < len(units):
            do_stage2(units[s2ptr[0]])
            s2ptr[0] += 1
    nc.compile()
    return nc


def _build_merge_nc(affine=True):
    global _ACT_KEEP
    _ACT_KEEP = {"sqrt_and_others"}
    nc = bacc.Bacc()
    ys0_d = nc.dram_tensor("ys0", [D, TH], H16, kind="ExternalInput")
    ys1_d = nc.dram_tensor("ys1", [D, TH], H16, kind="ExternalInput")
    w5_d = nc.dram_tensor("w5", [128, 25], H16, kind="ExternalInput")
    lnw_d = nc.dram_tensor("lnw", [1, D_INNER], F, kind="ExternalInput")
    lnb_d = nc.dram_tensor("lnb", [1, D_INNER], F, kind="ExternalInput")
    out_d = nc.dram_tensor("out", [TH, J, D_INNER], H16, kind="ExternalOutput")

    NJ = TH // 128  # 8 t-tiles

    with tile.TileContext(nc) as tc, ExitStack() as ctx:
        singles = ctx.enter_context(tc.tile_pool(name="singles", bufs=1))
        mpool = ctx.enter_context(tc.tile_pool(name="mpool", bufs=3))
        lpool = ctx.enter_context(tc.tile_pool(name="lpool", bufs=8))
        opool = ctx.enter_context(tc.tile_pool(name="opool", bufs=5))
        ptr = ctx.enter_context(tc.tile_pool(name="ptr", bufs=4, space="PSUM"))
        pstat = ctx.enter_context(tc.tile_pool(name="pstat", bufs=1, space="PSUM"))

        w5_t = singles.tile([128, 25], H16)
        nc.sync.dma_start(w5_t[:], w5_d[:, :])
        musd_t = singles.tile([64, TH], F)
        if affine:
            lnw_t = singles.tile([128, D_INNER], F)
            nc.sync.dma_start(lnw_t[:], bass.AP(
                tensor=lnw_d[:, :].tensor, offset=0, ap=[[0, 128], [1, D_INNER]]))
            lnb_t = singles.tile([128, D_INNER], F)
            nc.sync.dma_start(lnb_t[:], bass.AP(
                tensor=lnb_d[:, :].tensor, offset=0, ap=[[0, 128], [1, D_INNER]]))
        else:
            # weights are ones/zeros: keep the inputs referenced via a tiny load
            lnw_t = singles.tile([1, D_INNER], F)
            nc.sync.dma_start(lnw_t[:], lnw_d[:, :])
            lnb_t = singles.tile([1, D_INNER], F)
            nc.sync.dma_start(lnb_t[:], lnb_d[:, :])
        eps_t = singles.tile([128, 1], F)
        nc.vector.memset(eps_t[:], 1e-5)
        # tiny warm-up op so the ACT table load lands in the DMA lead-in
        # instead of delaying the first transpose copy
        warm_t = singles.tile([1, 1], F)
        nc.scalar.copy(warm_t[:], eps_t[0:1, :])
        ident = singles.tile([128, 128], H16)
        make_identity(nc, ident[:])
        identf = singles.tile([64, 64], F)
        make_identity(nc, identf[:])

        ymT = [singles.tile([128, D], F, name=f"ymT{j}", tag=f"ymT{j}") for j in range(NJ)]

        for i in range(NT):
            y0 = mpool.tile([128, TH], H16, tag="y0")
            nc.sync.dma_start(y0[:], ys0_d[i * 128:(i + 1) * 128, :])
            y1 = mpool.tile([128, TH], H16, tag="y1")
            nc.sync.dma_start(y1[:], ys1_d[i * 128:(i + 1) * 128, :])
            # the Dsum*u skip term is already folded in by the scan launch
            ym = mpool.tile([128, TH], H16, tag="ym")
            nc.gpsimd.tensor_add(ym[:], y0[:], y1[:])
            # LN stats on PE: per-joint sums of y and y^2 over the d-tiles
            sq = mpool.tile([128, TH], H16, tag="sq")
            nc.gpsimd.tensor_mul(sq[:], ym[:], ym[:])
            if i == 0:
                sy_ps = pstat.tile([J, TH], F, tag="sy", name="sy")
                sy2_ps = pstat.tile([J, TH], F, tag="sy2", name="sy2")
                stat_ps = [sy_ps, sy2_ps]
            wsl = w5_t[:, (i % 5) * J:(i % 5 + 1) * J]
            for srcm, dstp in ((ym, stat_ps[0]), (sq, stat_ps[1])):
                for c in range(TH // 512):
                    nc.tensor.matmul(
                        dstp[:, c * 512:(c + 1) * 512], wsl,
                        srcm[:, c * 512:(c + 1) * 512],
                        start=(i == 0), stop=(i == NT - 1))
            for jj in range(NJ):
                trp = ptr.tile([128, 128], H16, tag="trp")
                nc.tensor.transpose(trp[:], ym[:, jj * 128:(jj + 1) * 128], ident[:])
                if (i + jj) % 2:
                    nc.vector.tensor_copy(ymT[jj][:, i * 128:(i + 1) * 128], trp[:])
                else:
                    nc.scalar.copy(ymT[jj][:, i * 128:(i + 1) * 128], trp[:])

        # finish the stats: mu = sy/384, var = sy2/384 - mu^2, rsd = 1/sqrt
        muf_t = singles.tile([J, TH], F)
        nc.vector.tensor_scalar_mul(muf_t[:], stat_ps[0][:], 1.0 / D_INNER)
        muf = muf_t[:]
        m2f = singles.tile([J, TH], F)
        nc.vector.tensor_scalar_mul(m2f[:], stat_ps[1][:], 1.0 / D_INNER)
        mu2 = singles.tile([J, TH], F)
        nc.vector.tensor_mul(mu2[:], muf, muf)
        varf = singles.tile([J, TH], F)
        nc.vector.tensor_sub(varf[:], m2f[:], mu2[:])
        sdf = singles.tile([J, TH], F)
        nc.scalar.activation(sdf[:], varf[:], AF.Sqrt, bias=eps_t[0:J, :])
        rsdf = singles.tile([J, TH], F)
        nc.vector.reciprocal(rsdf[:], sdf[:])
        nc.gpsimd.tensor_copy(musd_t[0:J, :], muf)
        nc.gpsimd.tensor_copy(musd_t[32:32 + J, :], rsdf[:])

        for jj in range(NJ):
            trpm = ptr.tile([128, 64], F, tag="trp")
            nc.tensor.transpose(trpm[:], musd_t[:, jj * 128:(jj + 1) * 128],
                                identf[:])
            musdT = lpool.tile([128, 64], F, tag="musdT")
            nc.scalar.copy(musdT[:], trpm[:])
            for h in range(J):
                src = ymT[jj][:]
                yv = bass.AP(tensor=src.tensor, offset=src.offset + h,
                             ap=[src.ap[0], [J, D_INNER]])
                nrm = opool.tile([128, D_INNER], H16, tag="nrm")
                nc.vector.tensor_scalar(nrm[:], yv, musdT[:, h:h + 1],
                                        musdT[:, 32 + h:32 + h + 1],
                                        op0=OP.subtract, op1=OP.mult)
                if affine:
                    o1 = opool.tile([128, D_INNER], F, tag="o1")
                    nc.vector.tensor_mul(o1[:], nrm[:], lnw_t[:])
                    o2 = opool.tile([128, D_INNER], H16, tag="o2")
                    nc.vector.tensor_add(o2[:], o1[:], lnb_t[:])
                else:
                    o2 = nrm
                dst = bass.AP(tensor=out_d[:, :, :].tensor,
                              offset=jj * 128 * J * D_INNER + h * D_INNER,
                              ap=[[J * D_INNER, 128], [1, D_INNER]])
                engs = (nc.sync, nc.scalar)
                engs[(jj * J + h) % 2].dma_start(dst, o2[:])
    nc.compile()
    return nc


_CACHE = {}


def _get_ncs(affine=True):
    if "scan" not in _CACHE:
        _CACHE["scan"] = _build_scan_nc()
    mk = f"merge_{affine}"
    if mk not in _CACHE:
        _CACHE[mk] = _build_merge_nc(affine)
    return _CACHE["scan"], _CACHE[mk]


def kernel(x, x_proj_weight, dt_projs_weight, dt_projs_bias, A_logs, Ds,
           ln_weight, ln_bias):
    x = np.asarray(x, np.float32)
    affine = not (np.all(np.asarray(ln_weight) == 1.0)
                  and np.all(np.asarray(ln_bias) == 0.0))
    nc_scan, nc_merge = _get_ncs(affine)

    # host-side sharding prep (reshapes/transposes only)
    xflat = np.ascontiguousarray(
        np.transpose(np.asarray(x), (0, 1, 3, 2)).reshape(B, D, T))
    xflat16 = xflat.astype(np.float16)

    # scan-layout constant reshapes, per direction k: p = 2*d_loc + n_loc,
    # col = i*16 + hb*8 + g, d = i*128 + hb*64 + p//2, n = 2*g + p%2
    p = np.arange(128)
    cols = np.arange(NT * 2 * NG)
    ci, crem = cols // 16, cols % 16
    chb, cg = crem // 8, crem % 8
    d_idx = (ci[None, :] * 128 + chb[None, :] * 64 + (p // 2)[:, None])   # (128, C)
    n_idx = (cg[None, :] * 2 + (p % 2)[:, None])                          # (128, C)

    wx = np.asarray(x_proj_weight, np.float32)
    wxT_pad = np.zeros((K, D, 80), np.float32)
    for k in range(K):
        wxT_pad[k, :, 0:R] = wx[k, 0:R].T
        wxT_pad[k, :, 32:32 + N] = wx[k, R:R + N].T
        wxT_pad[k, :, 64:64 + N] = wx[k, R + N:R + 2 * N].T
    q = np.arange(128)
    s2 = np.zeros((128, 256), np.float32)
    for hb in range(2):
        s2[:, hb * 128:(hb + 1) * 128] = (
            np.arange(128)[None, :] == (hb * 64 + q // 2)[:, None])
    s2 = s2.astype(np.float16)

    ds_np0 = np.asarray(Ds, np.float32)
    dsum_r = np.ascontiguousarray(
        (ds_np0[0:D] + ds_np0[D:2 * D]).reshape(NT, 128).T)
    in_maps = []
    for c in range(8):
        b, k = c // 2, c % 2
        xk = xflat16[b] if k == 0 else xflat16[b][:, ::-1]
        al = np.asarray(A_logs, np.float32)[k * D:(k + 1) * D]            # (D, N)
        alog_r = al[d_idx, n_idx]                                          # (128, C)
        in_maps.append(dict(
            x=np.ascontiguousarray(xk),
            wxT=np.ascontiguousarray(
                wxT_pad[k].reshape(NT, 128, 80).transpose(1, 0, 2)
                .reshape(128, NT * 80)).astype(np.float16),
            wdtT=np.ascontiguousarray(np.asarray(dt_projs_weight, np.float32)[k].T).astype(np.float16),
            bias_r=np.ascontiguousarray(
                np.asarray(dt_projs_bias, np.float32)[k].reshape(NT, 128).T),
            alog_r=np.ascontiguousarray(alog_r),
            s2=s2,
            dsc=(dsum_r if k == 0 else np.zeros((128, NT), np.float32)),
        ))
    res1 = run_bass_kernel_spmd(nc_scan, in_maps, core_ids=list(range(8))).results

    lnw = np.asarray(ln_weight, np.float32).reshape(1, D_INNER)
    lnb = np.asarray(ln_bias, np.float32).reshape(1, D_INNER)
    w5 = np.zeros((128, 25), np.float32)
    qq = np.arange(128)
    for p5 in range(5):
        w5[qq, p5 * J + (p5 * 128 + qq) % 5] = 1.0
    w5 = w5.astype(np.float16)

    in_maps2 = []
    for c in range(8):
        b, th = c // 2, c % 2
        sl = slice(th * TH, (th + 1) * TH)
        ys0 = res1[2 * b]["ys"][:, sl]
        ys1 = res1[2 * b + 1]["ys"][:, ::-1][:, sl]
        in_maps2.append(dict(
            ys0=np.ascontiguousarray(ys0),
            ys1=np.ascontiguousarray(ys1),
            w5=w5, lnw=lnw, lnb=lnb,
        ))
    res2 = run_bass_kernel_spmd(nc_merge, in_maps2, core_ids=list(range(8))).results

    out = np.empty((B, T, J, D_INNER), np.float32)
    for c in range(8):
        b, th = c // 2, c % 2
        out[b, th * TH:(th + 1) * TH] = res2[c]["out"]
    return out
